# revision 31
# baseline (speedup 1.0000x reference)
"""Trainium2 Bass kernel for nn_AttentionBlock (B=4, S=2048, D=1024, H=16).

Sharding (8 cores): core c -> batch b = c//2, local heads = 8*(c%2) .. +8.
Phase 1 (uniform across cores): per-core QKV projection for its 8 heads over
all 2048 tokens, causal attention (transposed-scores layout, fused softmax
denominator via a ones-column in V), producing attn partial [2048, 512].
Exchange: pairwise ReduceScatter(add) over (2c, 2c+1) of a [2048, 1024]
tensor whose "other half" columns are zeroed via a per-core 0/1 input scalar,
yielding each core its 1024-token rows with all 1024 features.
Phase 2: x + attn -> LN1 -> +linear1 -> LN2 for the core's 1024 tokens.

Matmul operands are bf16 (PE 1 cyc/row + fast weight load); accumulation,
softmax denominators, residuals and layernorms stay fp32. Program is
identical on all 8 cores; only input data differs.
"""

import os
from contextlib import ExitStack

import numpy as np
import ml_dtypes

import concourse.bass as bass
import concourse.mybir as mybir
import concourse.tile as tile
from concourse import bacc
from concourse.masks import make_identity
from concourse.tile import add_dep_helper

F32 = mybir.dt.float32
BF16 = mybir.dt.bfloat16
AF = mybir.ActivationFunctionType
ALU = mybir.AluOpType

B, S, D = 4, 2048, 1024
H = 16
DH = 64
HL = 8            # local heads per core
DL = HL * DH      # 512: local projection width
P = 128           # partitions
CW = 512          # matmul N (one fp32 psum bank)
CW2 = 2 * CW      # paired-chunk width (two psum banks)
NKB = D // P      # 8 contraction blocks
NDB = DL // P     # 4 local dout blocks
NSB = S // P      # 16 token blocks
NCH = S // CW     # 4 query chunks
NCORES = 8
LN_EPS = 1e-5
SM_SCALE = 1.0 / 8.0  # 1/sqrt(DH)

REPLICA_GROUPS = [[0, 1], [2, 3], [4, 5], [6, 7]]


def _bcast_row(ap, parts=P):
    """AP view that broadcasts a [N] DRAM vector across `parts` partitions."""
    return bass.AP(tensor=ap.tensor, offset=ap.offset, ap=[[0, parts]] + list(ap.ap))


def build_program(use_pbias, use_vbias, use_g1, use_b1, use_lin1b, use_g2, use_b2):
    nc = bacc.Bacc("TRN2", num_devices=NCORES)

    xT_d = nc.declare_dram_parameter("xT", [D, S], BF16, isOutput=False)
    xq_d = nc.declare_dram_parameter("xq", [S // 2, D], F32, isOutput=False)
    wqT_d = nc.declare_dram_parameter("wqT", [D, DL], BF16, isOutput=False)
    wkT_d = nc.declare_dram_parameter("wkT", [D, DL], BF16, isOutput=False)
    wvT_d = nc.declare_dram_parameter("wvT", [D, DL], BF16, isOutput=False)
    lin1T_d = nc.declare_dram_parameter("lin1T", [D, D], BF16, isOutput=False)
    hm_d = nc.declare_dram_parameter("hmask", [P, 2], F32, isOutput=False)
    if use_pbias:
        qb_d = nc.declare_dram_parameter("qb", [P, NDB], F32, isOutput=False)
        kb_d = nc.declare_dram_parameter("kb", [P, NDB], F32, isOutput=False)
    if use_vbias:
        vb_d = nc.declare_dram_parameter("vb", [DL], F32, isOutput=False)
    if use_g1:
        ln1g_d = nc.declare_dram_parameter("ln1g", [D], F32, isOutput=False)
    if use_b1:
        ln1b_d = nc.declare_dram_parameter("ln1b", [D], F32, isOutput=False)
    if use_lin1b:
        lin1b_d = nc.declare_dram_parameter("lin1b", [D], F32, isOutput=False)
    if use_g2:
        ln2g_d = nc.declare_dram_parameter("ln2g", [D], F32, isOutput=False)
    if use_b2:
        ln2b_d = nc.declare_dram_parameter("ln2b", [D], F32, isOutput=False)
    out_d = nc.declare_dram_parameter("out", [S // 2, D], F32, isOutput=True)

    rsA = [nc.dram_tensor(f"rs_in_A{q}", [2 * P, D], BF16) for q in range(4)]
    rsB = [nc.dram_tensor(f"rs_in_B{q}", [2 * P, D], BF16) for q in range(4)]
    rsAo = [nc.dram_tensor(f"rs_out_A{q}", [P, D], BF16) for q in range(4)]
    rsBo = [nc.dram_tensor(f"rs_out_B{q}", [P, D], BF16) for q in range(4)]

    with tile.TileContext(nc) as tc, ExitStack() as ctx:
        singles = ctx.enter_context(tc.tile_pool(name="singles", bufs=1))
        ident = singles.tile([P, P], F32)
        make_identity(nc, ident[:])
        identb = singles.tile([P, P], BF16)
        nc.vector.tensor_copy(identb[:], ident[:])
        eps_t = singles.tile([P, 1], F32)
        nc.vector.memset(eps_t[:], LN_EPS)
        hm_t = singles.tile([P, 2], F32)
        nc.sync.dma_start(hm_t[:], hm_d[:])
        if use_vbias:
            vb_rep = singles.tile([P, DL], F32)
            nc.sync.dma_start(vb_rep[:], _bcast_row(vb_d[:]))
        ones8 = singles.tile([P, HL, 1], BF16)
        nc.vector.memset(ones8[:], 1.0)

        lin1_pool = ctx.enter_context(tc.tile_pool(name="lin1", bufs=NKB))
        lt = []
        for i in range(NKB):
            t = lin1_pool.tile([P, D], BF16, tag="lin1", name=f"lin1_{i}")
            nc.sync.dma_start(t[:], lin1T_d[i * P:(i + 1) * P, :])
            lt.append(t)

        kT_pool = ctx.enter_context(tc.tile_pool(name="kT", bufs=NDB))
        qT_pool = ctx.enter_context(tc.tile_pool(name="qT", bufs=NDB))
        vp_pool = ctx.enter_context(tc.tile_pool(name="vp", bufs=NSB))
        # one [128, 1024] fp32 pool (2 banks x 2 bufs) shared by projections,
        # scores and linear1
        mmps = ctx.enter_context(tc.tile_pool(name="mmps", bufs=2, space="PSUM"))

        kT_t = [kT_pool.tile([P, S], BF16, tag="kT", name=f"kT{i}")
                for i in range(NDB)]
        qT_t = [qT_pool.tile([P, S], BF16, tag="qT", name=f"qT{i}")
                for i in range(NDB)]
        vp_t = [vp_pool.tile([P, HL, DH + 1], BF16, tag="vp", name=f"vp{i}")
                for i in range(NSB)]

        # ---------------- projections ----------------
        with tc.tile_pool(name="xt", bufs=NKB) as xt_pool:
            xt = []
            for i in range(NKB):
                t = xt_pool.tile([P, S], BF16, tag="xt")
                nc.scalar.dma_start(t[:, 0:CW2], xT_d[i * P:(i + 1) * P, 0:CW2])
                xt.append(t)
            for i in range(NKB):
                nc.gpsimd.dma_start(xt[i][:, CW2:S],
                                    xT_d[i * P:(i + 1) * P, CW2:S])

            with tc.tile_pool(name="wpool", bufs=NKB) as w_pool, \
                 tc.tile_pool(name="bkq", bufs=1) as bkq_pool:
                if use_pbias:
                    kqb = bkq_pool.tile([P, 2 * NDB], F32)
                    nc.sync.dma_start(kqb[:, 0:NDB], kb_d[:])
                    nc.sync.dma_start(kqb[:, NDB:2 * NDB], qb_d[:])

                def project_kq(w_dram, dest, bcol0, eng):
                    wt = []
                    for i in range(NKB):
                        t = w_pool.tile([P, DL], BF16, tag="w",
                                        name=f"w_{w_dram.name}_{i}")
                        eng.dma_start(t[:], w_dram[i * P:(i + 1) * P, :])
                        wt.append(t)
                    for db in range(NDB):
                        for pc in range(S // CW2):  # paired 1024-wide chunks
                            ps = mmps.tile([P, CW2], F32, tag="mm", name="ps")
                            for half in range(2):
                                sc = 2 * pc + half
                                for kbi in range(NKB):
                                    nc.tensor.matmul(
                                        ps[:, half * CW:(half + 1) * CW],
                                        wt[kbi][:, db * P:(db + 1) * P],
                                        xt[kbi][:, sc * CW:(sc + 1) * CW],
                                        start=(kbi == 0), stop=(kbi == NKB - 1),
                                    )
                            dst = dest[db][:, pc * CW2:(pc + 1) * CW2]
                            if use_pbias:
                                nc.scalar.add(dst, ps[:],
                                              kqb[:, bcol0 + db:bcol0 + db + 1])
                            else:
                                nc.scalar.copy(dst, ps[:])

                project_kq(wkT_d, kT_t, 0, nc.sync)
                project_kq(wqT_d, qT_t, NDB, nc.gpsimd)

                # ---------------- v' projection ----------------
                wv = []
                for i in range(NKB):
                    t = w_pool.tile([P, DL], BF16, tag="w", name=f"w_wv_{i}")
                    nc.gpsimd.dma_start(t[:], wvT_d[i * P:(i + 1) * P, :])
                    wv.append(t)
                for spair in range(NSB // 2):
                    ps = mmps.tile([P, CW2], F32, tag="mm", name="psv")
                    for half in range(2):
                        sb = 2 * spair + half
                        for kbi in range(NKB):
                            nc.tensor.matmul(
                                ps[:, half * CW:(half + 1) * CW],
                                xt[kbi][:, sb * P:(sb + 1) * P],
                                wv[kbi][:],
                                start=(kbi == 0), stop=(kbi == NKB - 1),
                            )
                    for half in range(2):
                        sb = 2 * spair + half
                        pv = ps[:, half * CW:(half + 1) * CW].rearrange(
                            "p (h d) -> p h d", h=HL)
                        nc.scalar.copy(vp_t[sb][:, :, 0:DH], pv)
                        nc.scalar.copy(vp_t[sb][:, :, DH:DH + 1], ones8[:])

        # -------- attention + phase 2, interleaved schedule --------
        with tc.tile_pool(name="vecs", bufs=1) as vec_pool, \
             tc.tile_pool(name="cwork", bufs=5) as c_pool, \
             tc.tile_pool(name="hT", bufs=2 * NKB) as hT_pool, \
             tc.tile_pool(name="stat", bufs=8) as stat_pool, \
             tc.tile_pool(name="outp", bufs=3) as out_pool, \
             tc.tile_pool(name="apsum", bufs=2, space="PSUM") as apsum, \
             tc.tile_pool(name="tpsum", bufs=2, space="PSUM") as tpsum, \
             tc.tile_pool(name="probs", bufs=6) as probs_pool, \
             tc.tile_pool(name="band", bufs=4) as band_pool, \
             tc.tile_pool(name="atsb", bufs=3) as atsb_pool, \
             tc.tile_pool(name="attn_tm", bufs=2 * NCH) as attn_tm_pool, \
             tc.tile_pool(name="zst", bufs=4) as z_pool, \
             tc.tile_pool(name="small", bufs=12) as small_pool:

            reps = {}
            for name, flag, dram in (
                ("g1", use_g1, ln1g_d if use_g1 else None),
                ("b1", use_b1, ln1b_d if use_b1 else None),
                ("l1b", use_lin1b, lin1b_d if use_lin1b else None),
                ("g2", use_g2, ln2g_d if use_g2 else None),
                ("b2", use_b2, ln2b_d if use_b2 else None),
            ):
                if flag:
                    t = vec_pool.tile([P, D], F32, tag="vec_" + name)
                    nc.sync.dma_start(t[:], _bcast_row(dram[:]))
                    reps[name] = t

            def ln_apply(pre, gkey, bkey, pool, tag, dtype):
                st = stat_pool.tile([P, 2, 6], F32, tag="st")
                nc.vector.bn_stats(st[:, 0, :], pre[:, 0:CW])
                nc.vector.bn_stats(st[:, 1, :], pre[:, CW:D])
                mv = stat_pool.tile([P, 2], F32, tag="mv")
                nc.vector.bn_aggr(mv[:], st[:])
                sq = stat_pool.tile([P, 1], F32, tag="sq")
                nc.scalar.activation(sq[:], mv[:, 1:2], AF.Sqrt, bias=eps_t[:])
                rec = stat_pool.tile([P, 1], F32, tag="lrec")
                nc.vector.reciprocal(rec[:], sq[:])
                o = pool.tile([P, D], dtype, tag=tag)
                nc.vector.tensor_scalar(o[:], in0=pre[:], scalar1=mv[:, 0:1],
                                        scalar2=rec[:], op0=ALU.subtract,
                                        op1=ALU.mult)
                if gkey in reps:
                    nc.vector.tensor_mul(o[:], o[:], reps[gkey][:])
                if bkey in reps:
                    nc.vector.tensor_add(o[:], o[:], reps[bkey][:])
                return o

            def phaseC_block(sb, cc, hT_eng):
                rs_o = rsAo[sb] if sb < 4 else rsBo[sb - 4]
                xqt = c_pool.tile([P, D], F32, tag="xqt", name=f"xq{sb}")
                nc.sync.dma_start(xqt[:], xq_d[sb * P:(sb + 1) * P, :])
                att = c_pool.tile([P, D], BF16, tag="att", name=f"at{sb}")
                ld = nc.sync.dma_start(att[:], rs_o[:, :])
                add_dep_helper(ld.ins, cc.ins,
                               reason="phase2 reads ReduceScatter output")
                nc.vector.tensor_add(xqt[:], xqt[:], att[:])
                h_t = ln_apply(xqt, "g1", "b1", c_pool, "h", F32)
                hT = []
                for j in range(NKB):
                    tp = tpsum.tile([P, P], F32, tag="tpsum", name=f"htp{sb}_{j}")
                    nc.tensor.transpose(tp[:], h_t[:, j * P:(j + 1) * P],
                                        ident[:])
                    ht = hT_pool.tile([P, P], BF16, tag="hT")
                    if hT_eng == "act":
                        nc.scalar.copy(ht[:], tp[:])
                    else:
                        nc.vector.tensor_copy(ht[:], tp[:])
                    hT.append(ht)
                pl = mmps.tile([P, CW2], F32, tag="mm", name=f"pl{sb}")
                for dc in range(D // CW):
                    for kbi in range(NKB):
                        nc.tensor.matmul(
                            pl[:, dc * CW:(dc + 1) * CW],
                            hT[kbi][:],
                            lt[kbi][:, dc * CW:(dc + 1) * CW],
                            start=(kbi == 0), stop=(kbi == NKB - 1),
                        )
                if use_lin1b:
                    u = c_pool.tile([P, D], F32, tag="u", name=f"u{sb}")
                    nc.vector.tensor_add(u[:], pl[:], reps["l1b"][:])
                    nc.vector.tensor_add(h_t[:], u[:], h_t[:])
                else:
                    nc.vector.tensor_add(h_t[:], pl[:], h_t[:])
                o = ln_apply(h_t, "g2", "b2", out_pool, "out", F32)
                nc.sync.dma_start(out_d[sb * P:(sb + 1) * P, :], o[:])

            chunk_tm = {}

            def process_chunk_hp(s, hp):
                nfull = 4 * s
                npk = (nfull + 4) // 2
                if hp == 0:
                    chunk_tm[s] = [
                        attn_tm_pool.tile([P, DL], F32, tag="attn_tm",
                                          name=f"attn_tm_{s}_{i}")
                        for i in range(4)]
                attn_tm = chunk_tm[s]
                heads = (2 * hp, 2 * hp + 1)
                att_ps, band = {}, {}
                for h in heads:
                    att_ps[h] = apsum.tile([DH + 1, CW], F32, tag="apsum",
                                           name=f"aps_{s}_{h}")
                    band[h] = band_pool.tile([P, 4, CW], BF16, tag="band",
                                             name=f"band_{s}_{h}")
                kti = kT_t[hp]
                qti = qT_t[hp]

                def scores(h, kb, sc_ps, half):
                    poff = DH * (h % 2)
                    nc.tensor.matmul(
                        sc_ps[:, half * CW:(half + 1) * CW],
                        kti[poff:poff + DH, kb * P:(kb + 1) * P],
                        qti[poff:poff + DH, s * CW:(s + 1) * CW],
                        start=True, stop=True,
                    )

                # band blocks first: their masks (gpsimd) are ready early, so
                # later collectives on gpsimd never gate the PE
                for bp in range(2):
                    scps = {h: mmps.tile([P, CW2], F32, tag="mm",
                                         name=f"scb_{s}_{h}")
                            for h in heads}
                    for half in range(2):
                        for h in heads:  # alternate row halves: LDW overlaps
                            scores(h, nfull + 2 * bp + half, scps[h], half)
                    for h in heads:
                        nc.scalar.activation(
                            band[h][:, 2 * bp:2 * bp + 2, :], scps[h][:],
                            AF.Exp, scale=SM_SCALE)
                for h in heads:
                    nc.gpsimd.affine_select(
                        out=band[h][:], in_=band[h][:],
                        compare_op=ALU.is_ge, fill=0.0,
                        base=0, channel_multiplier=-1,
                        pattern=[[-P, 4], [1, CW]],
                    )

                def band_pvs(h, stop):
                    for j in range(4):
                        nc.tensor.matmul(
                            att_ps[h][:], vp_t[nfull + j][:, h, :],
                            band[h][:, j, :],
                            start=(j == 0), stop=(stop and j == 3),
                        )

                if nfull == 0:
                    for h in heads:
                        band_pvs(h, True)
                else:
                    pend = []
                    banddone = False
                    for pk in range(nfull // 2):
                        scps = {h: mmps.tile([P, CW2], F32, tag="mm",
                                             name=f"sc_{s}_{h}")
                                for h in heads}
                        for half in range(2):
                            for h in heads:
                                scores(h, 2 * pk + half, scps[h], half)
                        for h in heads:
                            pr = probs_pool.tile([P, CW2], BF16, tag="probs")
                            nc.scalar.activation(pr[:], scps[h][:], AF.Exp,
                                                 scale=SM_SCALE)
                            pend.append((h, pk, pr))
                        if not banddone:
                            # slack elapsed: masked band tiles ready now
                            for h in heads:
                                band_pvs(h, False)
                            banddone = True
                        while len(pend) > 2:
                            hh, ppk, pr = pend.pop(0)
                            for half in range(2):
                                kb = 2 * ppk + half
                                nc.tensor.matmul(
                                    att_ps[hh][:], vp_t[kb][:, hh, :],
                                    pr[:, half * CW:(half + 1) * CW],
                                    start=False, stop=False,
                                )
                    for hh, ppk, pr in pend:
                        for half in range(2):
                            kb = 2 * ppk + half
                            nc.tensor.matmul(
                                att_ps[hh][:], vp_t[kb][:, hh, :],
                                pr[:, half * CW:(half + 1) * CW],
                                start=False, stop=(kb == nfull - 1),
                            )
                for h in heads:
                    at_sb = atsb_pool.tile([DH + 1, CW], BF16, tag="atsb")
                    nc.vector.tensor_copy(at_sb[:], att_ps[h][:])
                    for qb4 in range(4):
                        tp = tpsum.tile([P, DH + 1], BF16, tag="tpsum",
                                        name=f"tp_{s}_{h}_{qb4}")
                        nc.tensor.transpose(
                            tp[:], at_sb[:, qb4 * P:(qb4 + 1) * P],
                            identb[0:DH + 1, 0:DH + 1])
                        rec = small_pool.tile([P, 1], F32, tag="rec")
                        nc.vector.reciprocal(rec[:], tp[:, DH:DH + 1])
                        dst = attn_tm[qb4][:, h * DH:(h + 1) * DH]
                        nc.vector.tensor_scalar_mul(dst, tp[:, 0:DH], rec[:])
                        if use_vbias:
                            nc.vector.tensor_add(
                                dst, dst, vb_rep[:, h * DH:(h + 1) * DH])

            # chunk s stages its 4 token-blocks into per-block exchange
            # tensors: chunks {0,2} -> rsA[q], {1,3} -> rsB[q]
            wr = {('A', q): [] for q in range(4)}
            wr.update({('B', q): [] for q in range(4)})

            def stage_chunk(s):
                key = 'A' if s in (0, 2) else 'B'
                row0 = 0 if s in (0, 1) else P
                tgt = rsA if key == 'A' else rsB
                for qb4 in range(4):
                    zA = z_pool.tile([P, DL], BF16, tag="zst")
                    zB = z_pool.tile([P, DL], BF16, tag="zst")
                    nc.vector.tensor_scalar_mul(zA[:], chunk_tm[s][qb4][:],
                                                hm_t[:, 0:1])
                    nc.vector.tensor_scalar_mul(zB[:], chunk_tm[s][qb4][:],
                                                hm_t[:, 1:2])
                    wr[(key, qb4)].append(
                        nc.sync.dma_start(tgt[qb4][row0:row0 + P, 0:DL], zA[:]))
                    wr[(key, qb4)].append(
                        nc.sync.dma_start(tgt[qb4][row0:row0 + P, DL:D], zB[:]))

            def collective(key, q):
                ins = (rsA if key == 'A' else rsB)[q]
                outs = (rsAo if key == 'A' else rsBo)[q]
                cc = nc.gpsimd.collective_compute(
                    "ReduceScatter", ALU.add,
                    replica_groups=REPLICA_GROUPS,
                    ins=[ins[:]], outs=[outs[:]],
                )
                for w in wr[(key, q)]:
                    add_dep_helper(cc.ins, w.ins,
                                   reason="rs waits for staged inputs")
                return cc

            # schedule: chunks 0,2,1 plain; chunk 3 interleaves the four A
            # collectives and phase-2 blocks 0..2; B collectives + remaining
            # blocks drain the tail.
            for s in (0, 2):
                for hp in range(HL // 2):
                    process_chunk_hp(s, hp)
                stage_chunk(s)
            for hp in range(HL // 2):
                process_chunk_hp(1, hp)
            stage_chunk(1)
            ccs = {}
            for hp in range(HL // 2):
                process_chunk_hp(3, hp)
                ccs[('A', hp)] = collective('A', hp)
                if hp >= 1:
                    phaseC_block(hp - 1, ccs[('A', hp - 1)], "dve")
            stage_chunk(3)
            phaseC_block(3, ccs[('A', 3)], "act")
            for q in range(4):
                ccs[('B', q)] = collective('B', q)
                phaseC_block(4 + q, ccs[('B', q)], "act")

    nc.compile()
    return nc


_PROG_CACHE = {}


def _get_prog(flags):
    if flags not in _PROG_CACHE:
        _PROG_CACHE[flags] = build_program(*flags)
    return _PROG_CACHE[flags]


def make_in_maps(x, wq_w, wq_b, wk_w, wk_b, wv_w, wv_b,
                 ln1_g, ln1_b, lin1_w, lin1_b, ln2_g, ln2_b, flags):
    use_pbias, use_vbias, use_g1, use_b1, use_lin1b, use_g2, use_b2 = flags
    f32 = np.float32
    bf16 = ml_dtypes.bfloat16
    xT = [np.ascontiguousarray(x[b].T.astype(bf16)) for b in range(B)]
    lin1T = np.ascontiguousarray(np.asarray(lin1_w, f32).T.astype(bf16))
    in_maps = []
    for c in range(NCORES):
        b, half = c // 2, c % 2
        rows = slice(DL * half, DL * (half + 1))
        m = {
            "xT": xT[b],
            "xq": np.ascontiguousarray(x[b, (S // 2) * half:(S // 2) * (half + 1)],
                                       dtype=f32),
            "wqT": np.ascontiguousarray(np.asarray(wq_w, f32)[rows].T.astype(bf16)),
            "wkT": np.ascontiguousarray(np.asarray(wk_w, f32)[rows].T.astype(bf16)),
            "wvT": np.ascontiguousarray(np.asarray(wv_w, f32)[rows].T.astype(bf16)),
            "lin1T": lin1T,
            "hmask": np.ascontiguousarray(
                np.tile(np.array([[1.0 - half, float(half)]], dtype=f32), (P, 1))),
        }
        if use_pbias:
            m["qb"] = np.ascontiguousarray(
                np.asarray(wq_b, f32)[rows].reshape(NDB, P).T)
            m["kb"] = np.ascontiguousarray(
                np.asarray(wk_b, f32)[rows].reshape(NDB, P).T)
        if use_vbias:
            m["vb"] = np.ascontiguousarray(np.asarray(wv_b, f32)[rows])
        if use_g1:
            m["ln1g"] = np.asarray(ln1_g, f32)
        if use_b1:
            m["ln1b"] = np.asarray(ln1_b, f32)
        if use_lin1b:
            m["lin1b"] = np.asarray(lin1_b, f32)
        if use_g2:
            m["ln2g"] = np.asarray(ln2_g, f32)
        if use_b2:
            m["ln2b"] = np.asarray(ln2_b, f32)
        in_maps.append(m)
    return in_maps


def compute_flags(wq_b, wk_b, wv_b, ln1_g, ln1_b, lin1_b, ln2_g, ln2_b):
    nz = lambda a: bool(np.any(np.asarray(a) != 0.0))
    return (
        nz(wq_b) or nz(wk_b),
        nz(wv_b),
        bool(np.any(np.asarray(ln1_g) != 1.0)),
        nz(ln1_b),
        nz(lin1_b),
        bool(np.any(np.asarray(ln2_g) != 1.0)),
        nz(ln2_b),
    )


def kernel(x, wq_w, wq_b, wk_w, wk_b, wv_w, wv_b,
           ln1_g, ln1_b, lin1_w, lin1_b, ln2_g, ln2_b):
    from concourse.bass_utils import run_bass_kernel_spmd

    x = np.asarray(x, np.float32)
    flags = compute_flags(wq_b, wk_b, wv_b, ln1_g, ln1_b, lin1_b, ln2_g, ln2_b)
    nc = _get_prog(flags)
    in_maps = make_in_maps(x, wq_w, wq_b, wk_w, wk_b, wv_w, wv_b,
                           ln1_g, ln1_b, lin1_w, lin1_b, ln2_g, ln2_b, flags)
    res = run_bass_kernel_spmd(nc, in_maps, list(range(NCORES)))
    out = np.empty((B, S, D), np.float32)
    for c in range(NCORES):
        b, half = c // 2, c % 2
        out[b, (S // 2) * half:(S // 2) * (half + 1)] = res.results[c]["out"]
    return out


# revision 34
# speedup vs baseline: 1.0134x; 1.0134x over previous
"""Trainium2 Bass kernel for nn_AttentionBlock (B=4, S=2048, D=1024, H=16).

Sharding (8 cores): core c -> batch b = c//2, local heads = 8*(c%2) .. +8.
Phase 1 (uniform across cores): per-core QKV projection for its 8 heads over
all 2048 tokens, causal attention (transposed-scores layout, fused softmax
denominator via a ones-column in V), producing attn partial [2048, 512].
Exchange: pairwise ReduceScatter(add) over (2c, 2c+1) of a [2048, 1024]
tensor whose "other half" columns are zeroed via a per-core 0/1 input scalar,
yielding each core its 1024-token rows with all 1024 features.
Phase 2: x + attn -> LN1 -> +linear1 -> LN2 for the core's 1024 tokens.

Matmul operands are bf16 (PE 1 cyc/row + fast weight load); accumulation,
softmax denominators, residuals and layernorms stay fp32. Program is
identical on all 8 cores; only input data differs.
"""

import os
from contextlib import ExitStack

import numpy as np
import ml_dtypes

import concourse.bass as bass
import concourse.mybir as mybir
import concourse.tile as tile
from concourse import bacc
from concourse.masks import make_identity
from concourse.tile import add_dep_helper


F32 = mybir.dt.float32
BF16 = mybir.dt.bfloat16
AF = mybir.ActivationFunctionType
ALU = mybir.AluOpType

B, S, D = 4, 2048, 1024
H = 16
DH = 64
HL = 8            # local heads per core
DL = HL * DH      # 512: local projection width
P = 128           # partitions
CW = 512          # matmul N (one fp32 psum bank)
CW2 = 2 * CW      # paired-chunk width (two psum banks)
NKB = D // P      # 8 contraction blocks
NDB = DL // P     # 4 local dout blocks
NSB = S // P      # 16 token blocks
NCH = S // CW     # 4 query chunks
NCORES = 8
LN_EPS = 1e-5
SM_SCALE = 1.0 / 8.0  # 1/sqrt(DH)

REPLICA_GROUPS = [[0, 1], [2, 3], [4, 5], [6, 7]]


def _bcast_row(ap, parts=P):
    """AP view that broadcasts a [N] DRAM vector across `parts` partitions."""
    return bass.AP(tensor=ap.tensor, offset=ap.offset, ap=[[0, parts]] + list(ap.ap))


def build_program(use_pbias, use_vbias, use_g1, use_b1, use_lin1b, use_g2, use_b2):
    nc = bacc.Bacc("TRN2", num_devices=NCORES)

    xT_d = nc.declare_dram_parameter("xT", [D, S], BF16, isOutput=False)
    xq_d = nc.declare_dram_parameter("xq", [S // 2, D], F32, isOutput=False)
    wqT_d = nc.declare_dram_parameter("wqT", [D, DL], BF16, isOutput=False)
    wkT_d = nc.declare_dram_parameter("wkT", [D, DL], BF16, isOutput=False)
    wvT_d = nc.declare_dram_parameter("wvT", [D, DL], BF16, isOutput=False)
    lin1T_d = nc.declare_dram_parameter("lin1T", [D, D], BF16, isOutput=False)
    hm_d = nc.declare_dram_parameter("hmask", [P, 2], F32, isOutput=False)
    if use_pbias:
        qb_d = nc.declare_dram_parameter("qb", [P, NDB], F32, isOutput=False)
        kb_d = nc.declare_dram_parameter("kb", [P, NDB], F32, isOutput=False)
    if use_vbias:
        vb_d = nc.declare_dram_parameter("vb", [DL], F32, isOutput=False)
    if use_g1:
        ln1g_d = nc.declare_dram_parameter("ln1g", [D], F32, isOutput=False)
    if use_b1:
        ln1b_d = nc.declare_dram_parameter("ln1b", [D], F32, isOutput=False)
    if use_lin1b:
        lin1b_d = nc.declare_dram_parameter("lin1b", [D], F32, isOutput=False)
    if use_g2:
        ln2g_d = nc.declare_dram_parameter("ln2g", [D], F32, isOutput=False)
    if use_b2:
        ln2b_d = nc.declare_dram_parameter("ln2b", [D], F32, isOutput=False)
    out_d = nc.declare_dram_parameter("out", [S // 2, D], F32, isOutput=True)

    rsA = [nc.dram_tensor(f"rs_in_A{q}", [2 * P, D], BF16) for q in range(4)]
    rsB = [nc.dram_tensor(f"rs_in_B{q}", [2 * P, D], BF16) for q in range(4)]
    rsAo = [nc.dram_tensor(f"rs_out_A{q}", [P, D], BF16) for q in range(4)]
    rsBo = [nc.dram_tensor(f"rs_out_B{q}", [P, D], BF16) for q in range(4)]

    with tile.TileContext(nc) as tc, ExitStack() as ctx:
        singles = ctx.enter_context(tc.tile_pool(name="singles", bufs=1))
        ident = singles.tile([P, P], F32)
        make_identity(nc, ident[:])
        identb = singles.tile([P, P], BF16)
        nc.vector.tensor_copy(identb[:], ident[:])
        eps_t = singles.tile([P, 1], F32)
        nc.vector.memset(eps_t[:], LN_EPS)
        hm_t = singles.tile([P, 2], F32)
        nc.sync.dma_start(hm_t[:], hm_d[:])
        if use_vbias:
            vb_rep = singles.tile([P, DL], F32)
            nc.sync.dma_start(vb_rep[:], _bcast_row(vb_d[:]))
        ones8 = singles.tile([P, HL, 1], BF16)
        nc.vector.memset(ones8[:], 1.0)

        lin1_pool = ctx.enter_context(tc.tile_pool(name="lin1", bufs=NKB))
        lt = []
        for i in range(NKB):
            t = lin1_pool.tile([P, D], BF16, tag="lin1", name=f"lin1_{i}")
            nc.sync.dma_start(t[:], lin1T_d[i * P:(i + 1) * P, :])
            lt.append(t)

        kT_pool = ctx.enter_context(tc.tile_pool(name="kT", bufs=NDB))
        qT_pool = ctx.enter_context(tc.tile_pool(name="qT", bufs=NDB))
        vp_pool = ctx.enter_context(tc.tile_pool(name="vp", bufs=NSB))
        # one [128, 1024] fp32 pool (2 banks x 2 bufs) shared by projections,
        # scores and linear1
        mmps = ctx.enter_context(tc.tile_pool(name="mmps", bufs=2, space="PSUM"))

        kT_t = [kT_pool.tile([P, S], BF16, tag="kT", name=f"kT{i}")
                for i in range(NDB)]
        qT_t = [qT_pool.tile([P, S], BF16, tag="qT", name=f"qT{i}")
                for i in range(NDB)]
        vp_t = [vp_pool.tile([P, HL, DH + 1], BF16, tag="vp", name=f"vp{i}")
                for i in range(NSB)]

        # ---------------- projections ----------------
        with tc.tile_pool(name="xt", bufs=NKB) as xt_pool:
            xt = []
            for i in range(NKB):
                t = xt_pool.tile([P, S], BF16, tag="xt")
                nc.scalar.dma_start(t[:, 0:CW2], xT_d[i * P:(i + 1) * P, 0:CW2])
                xt.append(t)
            for i in range(NKB):
                nc.gpsimd.dma_start(xt[i][:, CW2:S],
                                    xT_d[i * P:(i + 1) * P, CW2:S])

            with tc.tile_pool(name="wpool", bufs=NKB) as w_pool, \
                 tc.tile_pool(name="bkq", bufs=1) as bkq_pool:
                if use_pbias:
                    kqb = bkq_pool.tile([P, 2 * NDB], F32)
                    nc.sync.dma_start(kqb[:, 0:NDB], kb_d[:])
                    nc.sync.dma_start(kqb[:, NDB:2 * NDB], qb_d[:])

                def project_kq(w_dram, dest, bcol0, eng):
                    wt = []
                    for i in range(NKB):
                        t = w_pool.tile([P, DL], BF16, tag="w",
                                        name=f"w_{w_dram.name}_{i}")
                        eng.dma_start(t[:], w_dram[i * P:(i + 1) * P, :])
                        wt.append(t)
                    for db in range(NDB):
                        # kbi-outer: 4 consecutive matmuls share one stationary
                        pss = [mmps.tile([P, CW2], F32, tag="mm",
                                         name=f"ps{pc}") for pc in range(2)]
                        for kbi in range(NKB):
                            w_sl = wt[kbi][:, db * P:(db + 1) * P]
                            for pc in range(2):
                                for half in range(2):
                                    sc = 2 * pc + half
                                    nc.tensor.matmul(
                                        pss[pc][:, half * CW:(half + 1) * CW],
                                        w_sl,
                                        xt[kbi][:, sc * CW:(sc + 1) * CW],
                                        start=(kbi == 0), stop=(kbi == NKB - 1),
                                    )
                        for pc in range(2):
                            dst = dest[db][:, pc * CW2:(pc + 1) * CW2]
                            if use_pbias:
                                nc.scalar.add(dst, pss[pc][:],
                                              kqb[:, bcol0 + db:bcol0 + db + 1])
                            else:
                                nc.scalar.copy(dst, pss[pc][:])

                project_kq(wkT_d, kT_t, 0, nc.sync)
                project_kq(wqT_d, qT_t, NDB, nc.gpsimd)

                # ---------------- v' projection ----------------
                wv = []
                for i in range(NKB):
                    t = w_pool.tile([P, DL], BF16, tag="w", name=f"w_wv_{i}")
                    nc.gpsimd.dma_start(t[:], wvT_d[i * P:(i + 1) * P, :])
                    wv.append(t)
                for spair in range(NSB // 2):
                    ps = mmps.tile([P, CW2], F32, tag="mm", name="psv")
                    for half in range(2):
                        sb = 2 * spair + half
                        for kbi in range(NKB):
                            nc.tensor.matmul(
                                ps[:, half * CW:(half + 1) * CW],
                                xt[kbi][:, sb * P:(sb + 1) * P],
                                wv[kbi][:],
                                start=(kbi == 0), stop=(kbi == NKB - 1),
                            )
                    for half in range(2):
                        sb = 2 * spair + half
                        pv = ps[:, half * CW:(half + 1) * CW].rearrange(
                            "p (h d) -> p h d", h=HL)
                        nc.scalar.copy(vp_t[sb][:, :, 0:DH], pv)
                        nc.scalar.copy(vp_t[sb][:, :, DH:DH + 1], ones8[:])

        # -------- attention + phase 2, interleaved schedule --------
        with tc.tile_pool(name="vecs", bufs=1) as vec_pool, \
             tc.tile_pool(name="cwork", bufs=5) as c_pool, \
             tc.tile_pool(name="hT", bufs=2 * NKB) as hT_pool, \
             tc.tile_pool(name="stat", bufs=8) as stat_pool, \
             tc.tile_pool(name="outp", bufs=3) as out_pool, \
             tc.tile_pool(name="apsum", bufs=2, space="PSUM") as apsum, \
             tc.tile_pool(name="tpsum", bufs=2, space="PSUM") as tpsum, \
             tc.tile_pool(name="probs", bufs=6) as probs_pool, \
             tc.tile_pool(name="band", bufs=4) as band_pool, \
             tc.tile_pool(name="atsb", bufs=3) as atsb_pool, \
             tc.tile_pool(name="attn_tm", bufs=2 * NCH) as attn_tm_pool, \
             tc.tile_pool(name="zst", bufs=4) as z_pool, \
             tc.tile_pool(name="small", bufs=12) as small_pool:

            reps = {}
            for name, flag, dram in (
                ("g1", use_g1, ln1g_d if use_g1 else None),
                ("b1", use_b1, ln1b_d if use_b1 else None),
                ("l1b", use_lin1b, lin1b_d if use_lin1b else None),
                ("g2", use_g2, ln2g_d if use_g2 else None),
                ("b2", use_b2, ln2b_d if use_b2 else None),
            ):
                if flag:
                    t = vec_pool.tile([P, D], F32, tag="vec_" + name)
                    nc.sync.dma_start(t[:], _bcast_row(dram[:]))
                    reps[name] = t

            def ln_apply(pre, gkey, bkey, pool, tag, dtype, on_act=False):
                st = stat_pool.tile([P, 2, 6], F32, tag="st")
                nc.vector.bn_stats(st[:, 0, :], pre[:, 0:CW])
                nc.vector.bn_stats(st[:, 1, :], pre[:, CW:D])
                mv = stat_pool.tile([P, 2], F32, tag="mv")
                nc.vector.bn_aggr(mv[:], st[:])
                sq = stat_pool.tile([P, 1], F32, tag="sq")
                nc.scalar.activation(sq[:], mv[:, 1:2], AF.Sqrt, bias=eps_t[:])
                rec = stat_pool.tile([P, 1], F32, tag="lrec")
                nc.vector.reciprocal(rec[:], sq[:])
                o = pool.tile([P, D], dtype, tag=tag)
                if on_act:
                    nmr = stat_pool.tile([P, 1], F32, tag="nmr")
                    nc.vector.tensor_scalar(nmr[:], in0=mv[:, 0:1],
                                            scalar1=rec[:], scalar2=-1.0,
                                            op0=ALU.mult, op1=ALU.mult)
                    nc.scalar.activation(o[:], pre[:], AF.Identity,
                                         bias=nmr[:], scale=rec[:])
                else:
                    nc.vector.tensor_scalar(o[:], in0=pre[:],
                                            scalar1=mv[:, 0:1],
                                            scalar2=rec[:], op0=ALU.subtract,
                                            op1=ALU.mult)
                if gkey in reps:
                    nc.vector.tensor_mul(o[:], o[:], reps[gkey][:])
                if bkey in reps:
                    nc.vector.tensor_add(o[:], o[:], reps[bkey][:])
                return o

            def phaseC_block(sb, cc, hT_eng):
                rs_o = rsAo[sb] if sb < 4 else rsBo[sb - 4]
                xqt = c_pool.tile([P, D], F32, tag="xqt", name=f"xq{sb}")
                nc.sync.dma_start(xqt[:], xq_d[sb * P:(sb + 1) * P, :])
                att = c_pool.tile([P, D], BF16, tag="att", name=f"at{sb}")
                ld = nc.sync.dma_start(att[:], rs_o[:, :])
                add_dep_helper(ld.ins, cc.ins,
                               reason="phase2 reads ReduceScatter output")
                nc.vector.tensor_add(xqt[:], xqt[:], att[:])
                h_t = ln_apply(xqt, "g1", "b1", c_pool, "h", F32,
                               on_act=(hT_eng == "act"))
                hT = []
                for j in range(NKB):
                    tp = tpsum.tile([P, P], F32, tag="tpsum", name=f"htp{sb}_{j}")
                    nc.tensor.transpose(tp[:], h_t[:, j * P:(j + 1) * P],
                                        ident[:])
                    ht = hT_pool.tile([P, P], BF16, tag="hT")
                    if hT_eng == "act":
                        nc.scalar.copy(ht[:], tp[:])
                    else:
                        nc.vector.tensor_copy(ht[:], tp[:])
                    hT.append(ht)
                pl = mmps.tile([P, CW2], F32, tag="mm", name=f"pl{sb}")
                for kbi in range(NKB):
                    for dc in range(D // CW):
                        nc.tensor.matmul(
                            pl[:, dc * CW:(dc + 1) * CW],
                            hT[kbi][:],
                            lt[kbi][:, dc * CW:(dc + 1) * CW],
                            start=(kbi == 0), stop=(kbi == NKB - 1),
                        )
                if use_lin1b:
                    u = c_pool.tile([P, D], F32, tag="u", name=f"u{sb}")
                    nc.vector.tensor_add(u[:], pl[:], reps["l1b"][:])
                    nc.vector.tensor_add(h_t[:], u[:], h_t[:])
                else:
                    nc.vector.tensor_add(h_t[:], pl[:], h_t[:])
                o = ln_apply(h_t, "g2", "b2", out_pool, "out", F32,
                             on_act=(hT_eng == "act"))
                nc.sync.dma_start(out_d[sb * P:(sb + 1) * P, :], o[:])

            chunk_tm = {}

            def process_chunk_hp(s, hp):
                nfull = 4 * s
                npk = (nfull + 4) // 2
                if hp == 0:
                    chunk_tm[s] = [
                        attn_tm_pool.tile([P, DL], F32, tag="attn_tm",
                                          name=f"attn_tm_{s}_{i}")
                        for i in range(4)]
                attn_tm = chunk_tm[s]
                heads = (2 * hp, 2 * hp + 1)
                att_ps, band = {}, {}
                for h in heads:
                    att_ps[h] = apsum.tile([DH + 1, CW], F32, tag="apsum",
                                           name=f"aps_{s}_{h}")
                    band[h] = band_pool.tile([P, 4, CW], BF16, tag="band",
                                             name=f"band_{s}_{h}")
                kti = kT_t[hp]
                qti = qT_t[hp]

                def scores(h, kb, sc_ps, half):
                    poff = DH * (h % 2)
                    nc.tensor.matmul(
                        sc_ps[:, half * CW:(half + 1) * CW],
                        kti[poff:poff + DH, kb * P:(kb + 1) * P],
                        qti[poff:poff + DH, s * CW:(s + 1) * CW],
                        start=True, stop=True,
                    )

                # band blocks first: their masks (gpsimd) are ready early, so
                # later collectives on gpsimd never gate the PE
                for bp in range(2):
                    scps = {h: mmps.tile([P, CW2], F32, tag="mm",
                                         name=f"scb_{s}_{h}")
                            for h in heads}
                    for half in range(2):
                        for h in heads:  # alternate row halves: LDW overlaps
                            scores(h, nfull + 2 * bp + half, scps[h], half)
                    for h in heads:
                        nc.scalar.activation(
                            band[h][:, 2 * bp:2 * bp + 2, :], scps[h][:],
                            AF.Exp, scale=SM_SCALE)
                for h in heads:
                    nc.gpsimd.affine_select(
                        out=band[h][:], in_=band[h][:],
                        compare_op=ALU.is_ge, fill=0.0,
                        base=0, channel_multiplier=-1,
                        pattern=[[-P, 4], [1, CW]],
                    )

                def band_pvs(h, stop):
                    for j in range(4):
                        nc.tensor.matmul(
                            att_ps[h][:], vp_t[nfull + j][:, h, :],
                            band[h][:, j, :],
                            start=(j == 0), stop=(stop and j == 3),
                        )

                if nfull == 0:
                    for h in heads:
                        band_pvs(h, True)
                else:
                    pend = []
                    banddone = False
                    for pk in range(nfull // 2):
                        scps = {h: mmps.tile([P, CW2], F32, tag="mm",
                                             name=f"sc_{s}_{h}")
                                for h in heads}
                        for half in range(2):
                            for h in heads:
                                scores(h, 2 * pk + half, scps[h], half)
                        for h in heads:
                            pr = probs_pool.tile([P, CW2], BF16, tag="probs")
                            nc.scalar.activation(pr[:], scps[h][:], AF.Exp,
                                                 scale=SM_SCALE)
                            pend.append((h, pk, pr))
                        if not banddone:
                            # slack elapsed: masked band tiles ready now
                            for h in heads:
                                band_pvs(h, False)
                            banddone = True
                        while len(pend) > 2:
                            hh, ppk, pr = pend.pop(0)
                            for half in range(2):
                                kb = 2 * ppk + half
                                nc.tensor.matmul(
                                    att_ps[hh][:], vp_t[kb][:, hh, :],
                                    pr[:, half * CW:(half + 1) * CW],
                                    start=False, stop=False,
                                )
                    for hh, ppk, pr in pend:
                        for half in range(2):
                            kb = 2 * ppk + half
                            nc.tensor.matmul(
                                att_ps[hh][:], vp_t[kb][:, hh, :],
                                pr[:, half * CW:(half + 1) * CW],
                                start=False, stop=(kb == nfull - 1),
                            )
                for h in heads:
                    at_sb = atsb_pool.tile([DH + 1, CW], BF16, tag="atsb")
                    nc.vector.tensor_copy(at_sb[:], att_ps[h][:])
                    for qb4 in range(4):
                        tp = tpsum.tile([P, DH + 1], BF16, tag="tpsum",
                                        name=f"tp_{s}_{h}_{qb4}")
                        nc.tensor.transpose(
                            tp[:], at_sb[:, qb4 * P:(qb4 + 1) * P],
                            identb[0:DH + 1, 0:DH + 1])
                        rec = small_pool.tile([P, 1], F32, tag="rec")
                        nc.vector.reciprocal(rec[:], tp[:, DH:DH + 1])
                        dst = attn_tm[qb4][:, h * DH:(h + 1) * DH]
                        nc.vector.tensor_scalar_mul(dst, tp[:, 0:DH], rec[:])
                        if use_vbias:
                            nc.vector.tensor_add(
                                dst, dst, vb_rep[:, h * DH:(h + 1) * DH])

            # chunk s stages its 4 token-blocks into per-block exchange
            # tensors: chunks {0,2} -> rsA[q], {1,3} -> rsB[q]
            wr = {('A', q): [] for q in range(4)}
            wr.update({('B', q): [] for q in range(4)})

            def stage_chunk(s):
                key = 'A' if s in (0, 2) else 'B'
                row0 = 0 if s in (0, 1) else P
                tgt = rsA if key == 'A' else rsB
                for qb4 in range(4):
                    zA = z_pool.tile([P, DL], BF16, tag="zst")
                    zB = z_pool.tile([P, DL], BF16, tag="zst")
                    nc.vector.tensor_scalar_mul(zA[:], chunk_tm[s][qb4][:],
                                                hm_t[:, 0:1])
                    nc.vector.tensor_scalar_mul(zB[:], chunk_tm[s][qb4][:],
                                                hm_t[:, 1:2])
                    wr[(key, qb4)].append(
                        nc.sync.dma_start(tgt[qb4][row0:row0 + P, 0:DL], zA[:]))
                    wr[(key, qb4)].append(
                        nc.sync.dma_start(tgt[qb4][row0:row0 + P, DL:D], zB[:]))

            def collective(key, q):
                ins = (rsA if key == 'A' else rsB)[q]
                outs = (rsAo if key == 'A' else rsBo)[q]
                cc = nc.gpsimd.collective_compute(
                    "ReduceScatter", ALU.add,
                    replica_groups=REPLICA_GROUPS,
                    ins=[ins[:]], outs=[outs[:]],
                )
                for w in wr[(key, q)]:
                    add_dep_helper(cc.ins, w.ins,
                                   reason="rs waits for staged inputs")
                return cc

            # schedule: chunks 0,2,1 plain; chunk 3 interleaves the four A
            # collectives and phase-2 blocks 0..2; B collectives + remaining
            # blocks drain the tail.
            for s in (0, 2):
                for hp in range(HL // 2):
                    process_chunk_hp(s, hp)
                stage_chunk(s)
            for hp in range(HL // 2):
                process_chunk_hp(1, hp)
            stage_chunk(1)
            ccs = {}
            for hp in range(HL // 2):
                process_chunk_hp(3, hp)
                ccs[('A', hp)] = collective('A', hp)
                if hp >= 1:
                    phaseC_block(hp - 1, ccs[('A', hp - 1)], "dve")
            stage_chunk(3)
            phaseC_block(3, ccs[('A', 3)], "act")
            for q in range(4):
                ccs[('B', q)] = collective('B', q)
                phaseC_block(4 + q, ccs[('B', q)], "act")

    nc.compile()
    return nc


_PROG_CACHE = {}


def _get_prog(flags):
    if flags not in _PROG_CACHE:
        _PROG_CACHE[flags] = build_program(*flags)
    return _PROG_CACHE[flags]


def make_in_maps(x, wq_w, wq_b, wk_w, wk_b, wv_w, wv_b,
                 ln1_g, ln1_b, lin1_w, lin1_b, ln2_g, ln2_b, flags):
    use_pbias, use_vbias, use_g1, use_b1, use_lin1b, use_g2, use_b2 = flags
    f32 = np.float32
    bf16 = ml_dtypes.bfloat16
    xT = [np.ascontiguousarray(x[b].T.astype(bf16)) for b in range(B)]
    lin1T = np.ascontiguousarray(np.asarray(lin1_w, f32).T.astype(bf16))
    in_maps = []
    for c in range(NCORES):
        b, half = c // 2, c % 2
        rows = slice(DL * half, DL * (half + 1))
        m = {
            "xT": xT[b],
            "xq": np.ascontiguousarray(x[b, (S // 2) * half:(S // 2) * (half + 1)],
                                       dtype=f32),
            "wqT": np.ascontiguousarray(np.asarray(wq_w, f32)[rows].T.astype(bf16)),
            "wkT": np.ascontiguousarray(np.asarray(wk_w, f32)[rows].T.astype(bf16)),
            "wvT": np.ascontiguousarray(np.asarray(wv_w, f32)[rows].T.astype(bf16)),
            "lin1T": lin1T,
            "hmask": np.ascontiguousarray(
                np.tile(np.array([[1.0 - half, float(half)]], dtype=f32), (P, 1))),
        }
        if use_pbias:
            m["qb"] = np.ascontiguousarray(
                np.asarray(wq_b, f32)[rows].reshape(NDB, P).T)
            m["kb"] = np.ascontiguousarray(
                np.asarray(wk_b, f32)[rows].reshape(NDB, P).T)
        if use_vbias:
            m["vb"] = np.ascontiguousarray(np.asarray(wv_b, f32)[rows])
        if use_g1:
            m["ln1g"] = np.asarray(ln1_g, f32)
        if use_b1:
            m["ln1b"] = np.asarray(ln1_b, f32)
        if use_lin1b:
            m["lin1b"] = np.asarray(lin1_b, f32)
        if use_g2:
            m["ln2g"] = np.asarray(ln2_g, f32)
        if use_b2:
            m["ln2b"] = np.asarray(ln2_b, f32)
        in_maps.append(m)
    return in_maps


def compute_flags(wq_b, wk_b, wv_b, ln1_g, ln1_b, lin1_b, ln2_g, ln2_b):
    nz = lambda a: bool(np.any(np.asarray(a) != 0.0))
    return (
        nz(wq_b) or nz(wk_b),
        nz(wv_b),
        bool(np.any(np.asarray(ln1_g) != 1.0)),
        nz(ln1_b),
        nz(lin1_b),
        bool(np.any(np.asarray(ln2_g) != 1.0)),
        nz(ln2_b),
    )


def kernel(x, wq_w, wq_b, wk_w, wk_b, wv_w, wv_b,
           ln1_g, ln1_b, lin1_w, lin1_b, ln2_g, ln2_b):
    from concourse.bass_utils import run_bass_kernel_spmd

    x = np.asarray(x, np.float32)
    flags = compute_flags(wq_b, wk_b, wv_b, ln1_g, ln1_b, lin1_b, ln2_g, ln2_b)
    nc = _get_prog(flags)
    in_maps = make_in_maps(x, wq_w, wq_b, wk_w, wk_b, wv_w, wv_b,
                           ln1_g, ln1_b, lin1_w, lin1_b, ln2_g, ln2_b, flags)
    res = run_bass_kernel_spmd(nc, in_maps, list(range(NCORES)))
    out = np.empty((B, S, D), np.float32)
    for c in range(NCORES):
        b, half = c // 2, c % 2
        out[b, (S // 2) * half:(S // 2) * (half + 1)] = res.results[c]["out"]
    return out
